# revision 73
# baseline (speedup 1.0000x reference)
"""Trainium2 Bass kernel for nn_Attention_33741263077435.

Reference computation (per batch b):
  q/k/v = conv2d_3x3(x, w{q,k,v}) + b{q,k,v}   (C=64 -> Cd=32, per frame s)
  attn  = sigmoid((q @ k^T) / 32)  per (b, channel)    (S=64, f=H*W)
  out   = attn @ v
  y     = conv2d_3x3(out, wo) + bo             (Cd=32 -> C=64)

Sharding: data-parallel over batch B=16 across 8 cores (2 batch elems/core).

Design (cost model: matmul cost = N_free rows x 0.42ns; K and M are free):
  - Frames are stored as a contiguous 36-wide scan (32 data cols + 4 zero
    pad cols shared between rows; 1152 = 9*128 scan px/frame). Both convs
    run "flipped": stationary = scan window [K=(ch,shift-copy), M=128
    contiguous scan px], moving = weights [K, N=ch_out]. A contiguous
    window is a legal 1-free-dim stationary AP, M=128 is full, and the
    shared pads make edge outputs exact.
  - Conv1 runs 5 passes per (frame, chunk): 3 row-pair passes on the
    [A;A+36] tile (dy=-1&0 x dx), 1 col-pair pass on [A+72;A+73]
    (dy=+1, dx=-1&0), 1 K=64 single (dy=+1,dx=+1): 553K PE cyc/core
    (direct normal-orientation formulation: 786K). Conv3 uses one K=96
    pass per dx over 3 row-shifted copies: 221K cyc (was 393K).
  - The 4 phantom px per row are zeroed during conv1 evacuation via the
    per-partition mask operand of scalar_tensor_tensor (out = ps*mask +
    mask*bias); zero phantoms in q/k/v are exact for attention (logits
    sum them as 0) and phantom columns of attn-out are exactly the zero
    pad cols conv3 needs.
  - Conv1 evacuations (DVE) write s-strided columns directly into an
    SBUF-resident qkv staging tile [128 f-scan, 9 j, 96 ch, 64 s]: no HBM
    qkv round trip and no q/k PE transposes (logits matmuls read staging
    views directly).
  - attn@v uses a block-diagonal att stationary [(2c,t), (2c,s)] so M=128
    covers a whole channel pair in 3 matmuls; v is pivoted to [(c,s), f]
    via 9 PE transposes/pair. Attention pairs are software-pipelined
    (attn@v of pair i emitted after logits of pair i+1) so PE never waits
    on the sigmoid.
  - Flipped conv3 writes a pixel-major bf16 HBM layout [b, g, px, j, f,
    C]; the host untransposes, casts to fp32 and adds the output bias.
  - x is loaded bf16 from a host-prepadded scan buffer (4 shifted window
    copies per group); GPSIMD cannot touch PSUM, so all PSUM evacuations
    sit on DVE/ACT while Pool handles SWDGE output/attn-staging writes
    (keeping stores off the load queues avoids head-of-line blocking).
  - Emission order: conv1(b0); attn(b0); conv1(b1) with 10 conv3(b0)
    groups hoisted into its DMA slack; remaining conv3(b0)+attn(b1)
    interleaved; conv3(b1). Per-batch-elem attn staging tensors avoid
    false whole-tensor RAW deps.
"""

import os
import sys

import numpy as np

for _p in ("/opt/trn_rl_repo", "/root/.axon_site/_ro/trn_rl_repo"):
    if os.path.isdir(_p) and _p not in sys.path:
        sys.path.append(_p)

import concourse.bass as bass  # noqa: E402
import concourse.tile as tile  # noqa: E402
from concourse import bacc, mybir  # noqa: E402
from concourse.bass_utils import run_bass_kernel_spmd  # noqa: E402

F32 = mybir.dt.float32
BF16 = mybir.dt.bfloat16

B, C, S, H, W = 16, 64, 64, 32, 32
Cd = C // 2
NCORES = 8
BL = B // NCORES
SCALE = 1.0 / np.sqrt(H * W)
FR = 4    # frames per group
NG = S // FR  # 16 groups
WS = 36   # scan row width (32 data + 4 shared zero pads)
FSC = WS * H        # 1152 scan px per frame
NJ = FSC // 128     # 9 chunks of 128 scan px
L1 = 1264           # conv1 source scan length per frame (with margins)
LP = 1228           # conv1 pad tile cols (window reads < 1228)
L3 = 1226           # attn scan staging length (37 + 1152 + 37)
LP3 = 1154          # conv3 pad tile cols
SIG = mybir.ActivationFunctionType.Sigmoid
ADD = mybir.AluOpType.add
MULT = mybir.AluOpType.mult


def build_kernel():
    nc = bacc.Bacc("TRN2", target_bir_lowering=False, debug=False)

    xs = nc.dram_tensor("xs", [BL, C, S, L1], BF16, kind="ExternalInput")
    wpair = nc.dram_tensor("wpair", [128, 3, 3 * Cd], BF16, kind="ExternalInput")
    wpair2 = nc.dram_tensor("wpair2", [128, 3 * Cd], BF16, kind="ExternalInput")
    wsing = nc.dram_tensor("wsing", [64, 3, 3 * Cd], BF16, kind="ExternalInput")
    maskc = nc.dram_tensor("maskc", [128, NJ], F32, kind="ExternalInput")
    biasj = nc.dram_tensor("biasj", [128, NJ, 3 * Cd, FR], BF16,
                           kind="ExternalInput")
    wo3 = nc.dram_tensor("wo3", [96, 3, C], BF16, kind="ExternalInput")
    identb = nc.dram_tensor("identb", [128, 128], BF16, kind="ExternalInput")

    # pixel-major output: [b, group, px, j, f, C]; host untransposes
    out_d = nc.dram_tensor("out", [BL, NG, 128, NJ, FR, C], BF16,
                           kind="ExternalOutput")
    # one staging tensor per batch elem: avoids false whole-tensor deps
    # between attn(b1) writes and conv3(b0) reads
    attn_ds = [nc.dram_tensor(f"attn_st{b}", [Cd, S, L3], BF16, kind="Internal")
               for b in range(BL)]

    with tile.TileContext(nc) as tc:
        from contextlib import ExitStack

        with ExitStack() as ctx:
            consts = ctx.enter_context(tc.tile_pool(name="consts", bufs=1))
            wp_sb = consts.tile([128, 3, 3 * Cd], BF16)
            nc.gpsimd.dma_start(wp_sb[:], wpair[:, :, :])
            wp2_sb = consts.tile([128, 3 * Cd], BF16)
            nc.gpsimd.dma_start(wp2_sb[:], wpair2[:, :])
            ws_sb = consts.tile([64, 3, 3 * Cd], BF16)
            nc.gpsimd.dma_start(ws_sb[:], wsing[:, :, :])
            mask_sb = consts.tile([128, NJ], F32)
            nc.gpsimd.dma_start(mask_sb[:], maskc[:, :])
            bj_sb = consts.tile([128, NJ, 3 * Cd, FR], BF16)
            nc.gpsimd.dma_start(bj_sb[:], biasj[:, :, :, :])
            wo_sb = consts.tile([96, 3, C], BF16)
            nc.gpsimd.dma_start(wo_sb[:], wo3[:, :, :])
            idb_sb = consts.tile([128, 128], BF16)
            nc.gpsimd.dma_start(idb_sb[:], identb[:, :])

            # qkv staging: [128 scan-part, j, ch(q|k|v), s] bf16, SBUF-resident
            stage = ctx.enter_context(tc.tile_pool(name="stage", bufs=1))
            QKV = stage.tile([128, NJ, 3 * Cd, S], BF16)
            # block-diag att tiles: off-diag blocks zeroed once, reused
            att_bufs = [stage.tile([128, 128], BF16, name=f"attbd{i}")
                        for i in range(2)]
            for a in att_bufs:
                nc.gpsimd.memset(a[0:64, 64:128], 0.0)
                nc.gpsimd.memset(a[64:128, 0:64], 0.0)
            # attn evac tiles with persistent zero margins
            evO_bufs = [stage.tile([128, L3], BF16, name=f"evo{i}")
                        for i in range(2)]
            for a in evO_bufs:
                nc.gpsimd.memset(a[:, 0:37], 0.0)
                nc.gpsimd.memset(a[:, 37 + FSC:L3], 0.0)

            with (
                tc.tile_pool(name="pad1", bufs=2) as pad_pool,
                tc.tile_pool(name="pad2", bufs=2) as pad2_pool,
                tc.tile_pool(name="psc", bufs=3, space="PSUM") as ps_pool,
                tc.tile_pool(name="v2", bufs=2) as v2_pool,
                tc.tile_pool(name="psT", bufs=2, space="PSUM") as psT_pool,
                tc.tile_pool(name="psA", bufs=1, space="PSUM") as psA_pool,
                tc.tile_pool(name="psO", bufs=2, space="PSUM") as psO_pool,
                tc.tile_pool(name="pad3", bufs=3) as pad3_pool,
                tc.tile_pool(name="oev", bufs=3) as oev_pool,
            ):

                def conv1_group(b, g):
                    s0 = FR * g
                    pad = pad_pool.tile([128, FR, LP3], BF16)
                    # A half (parts 0-63): pad[i] = xscan[i-37]  (dy=-1 role)
                    nc.sync.dma_start(pad[0:64, :, :],
                                      xs[b, :, s0:s0 + FR, 0:LP3])
                    # B half: pad[i] = xscan[i-1]  (dy=0 role)
                    nc.sync.dma_start(pad[64:128, :, :],
                                      xs[b, :, s0:s0 + FR, 36:36 + LP3])
                    # pad2: [A+72 ; A+73] -> pair (dy=+1,dx=-1)&(dy=+1,dx=0)
                    pad2 = pad2_pool.tile([128, FR, LP3], BF16)
                    nc.sync.dma_start(pad2[0:64, :, :],
                                      xs[b, :, s0:s0 + FR, 72:72 + LP3])
                    nc.sync.dma_start(pad2[64:128, :, :],
                                      xs[b, :, s0:s0 + FR, 73:73 + LP3])
                    for j in range(NJ):
                        m0 = 128 * j
                        ps = ps_pool.tile([128, FR, 3 * Cd], F32, tag="conv")
                        for f in range(FR):
                            for i in range(3):
                                # pair pass: A->dy=-1, B->dy=0, dx=i-1
                                nc.tensor.matmul(
                                    ps[:, f, :],
                                    pad[0:128, f, m0 + i:m0 + i + 128],
                                    wp_sb[:, i, :],
                                    start=(i == 0), stop=False,
                                )
                            # pair pass: dy=+1, dx=-1 & dx=0
                            nc.tensor.matmul(
                                ps[:, f, :],
                                pad2[0:128, f, m0:m0 + 128],
                                wp2_sb[:, :],
                                start=False, stop=False,
                            )
                            # single: dy=+1, dx=+1 (pad2 A-half at +2)
                            nc.tensor.matmul(
                                ps[:, f, :],
                                pad2[0:64, f, m0 + 2:m0 + 130],
                                ws_sb[:, 2, :],
                                start=False, stop=True,
                            )
                        nc.vector.scalar_tensor_tensor(
                            QKV[:, j, :, s0:s0 + FR],
                            ps[:, :, :].rearrange("p f n -> p n f"),
                            mask_sb[:, j:j + 1],
                            bj_sb[:, j, :, :],
                            op0=MULT, op1=ADD,
                        )

                attn_v2 = {}

                def attn_part1(b, cp):
                    c0 = 2 * cp
                    # v pivot: [f', (c,s)] -> [(c,s), f'] via PE transposes
                    v2 = v2_pool.tile([128, FSC], BF16)
                    for hf in range(3):
                        nch = 4 if hf < 2 else 1
                        psT = psT_pool.tile([128, 512], BF16, tag="psT")
                        for i in range(nch):
                            j = 4 * hf + i
                            nc.tensor.transpose(
                                psT[:, 128 * i:128 * i + 128],
                                QKV[:, j, 2 * Cd + c0:2 * Cd + c0 + 2, :],
                                idb_sb[:],
                            )
                        nc.vector.tensor_copy(
                            v2[:, 512 * hf:512 * hf + 128 * nch],
                            psT[:, 0:128 * nch])
                    # logits^T accumulate over f' chunks
                    psA = psA_pool.tile([128, 128], F32)
                    for j in range(NJ):
                        nc.tensor.matmul(
                            psA[:],
                            QKV[:, j, Cd + c0:Cd + c0 + 2, :],
                            QKV[:, j, c0:c0 + 2, :],
                            start=(j == 0), stop=(j == NJ - 1),
                        )
                    att = att_bufs[cp % 2]
                    nc.scalar.activation(att[0:64, 0:64], psA[0:64, 0:64], SIG,
                                         scale=float(SCALE))
                    nc.scalar.activation(att[64:128, 64:128], psA[64:128, 64:128],
                                         SIG, scale=float(SCALE))
                    attn_v2[(b, cp)] = v2

                def attn_part2(b, cp):
                    c0 = 2 * cp
                    v2 = attn_v2.pop((b, cp))
                    att = att_bufs[cp % 2]
                    evO = evO_bufs[cp % 2]
                    for t3 in range(3):
                        psO = psO_pool.tile([128, 384], F32)
                        nc.tensor.matmul(
                            psO[:], att[:, :], v2[:, 384 * t3:384 * t3 + 384],
                            start=True, stop=True,
                        )
                        nc.scalar.add(
                            evO[:, 37 + 384 * t3:37 + 384 * t3 + 384],
                            psO[:], 0.0)
                    nc.gpsimd.dma_start(
                        attn_ds[b][c0:c0 + 2, :, :].rearrange("c s l -> (c s) l"),
                        evO[:],
                    )

                def conv3_group(b, g):
                    s0 = FR * g
                    pad3 = pad3_pool.tile([96, FR, LP3], BF16)
                    src = attn_ds[b][:, s0:s0 + FR, :]
                    nc.sync.dma_start(pad3[0:32, :, :], src[:, :, 0:LP3])
                    nc.sync.dma_start(pad3[32:64, :, :], src[:, :, 36:36 + LP3])
                    nc.sync.dma_start(pad3[64:96, :, :], src[:, :, 72:72 + LP3])
                    oev = oev_pool.tile([128, NJ, FR, C], BF16)
                    for j in range(NJ):
                        m0 = 128 * j
                        ps = ps_pool.tile([128, FR, C], F32, tag="conv")
                        for f in range(FR):
                            for i in range(3):
                                nc.tensor.matmul(
                                    ps[:, f, :],
                                    pad3[0:96, f, m0 + i:m0 + i + 128],
                                    wo_sb[:, i, :],
                                    start=(i == 0), stop=(i == 2),
                                )
                        # bias bo is added on the host during un-transpose
                        if j % 2 == 0:
                            nc.scalar.add(oev[:, j, :, :], ps[:, :, :], 0.0)
                        else:
                            nc.vector.tensor_copy(oev[:, j, :, :], ps[:, :, :])
                    # Pool/SWDGE queue: empty, so waiting on oev evacs here
                    # does not head-block any loads
                    nc.gpsimd.dma_start(out_d[b, g], oev[:, :, :, :])

                NP = Cd // 2  # 16 channel pairs
                for g in range(NG):
                    conv1_group(0, g)
                attn_part1(0, 0)
                for cp in range(1, NP):
                    attn_part1(0, cp)
                    attn_part2(0, cp - 1)
                attn_part2(0, NP - 1)
                # hoist 4 conv3(b0) groups into the PE-bound conv1(b1)
                # phase (its DMA has slack); rest interleave with attn(b1)
                HOIST = 10
                hoist_at = {1: 0, 3: 1, 4: 2, 6: 3, 8: 4, 9: 5, 11: 6, 12: 7,
                            14: 8, 15: 9}
                for g in range(NG):
                    conv1_group(1, g)
                    if g in hoist_at:
                        conv3_group(0, hoist_at[g])
                # spread the remaining conv3(b0) groups across the 16
                # attn(b1) pair slots
                d_at = {0: 10, 3: 11, 6: 12, 9: 13, 12: 14, 15: 15}
                if 0 in d_at:
                    conv3_group(0, d_at[0])
                attn_part1(1, 0)
                for i in range(1, NG):
                    if i in d_at:
                        conv3_group(0, d_at[i])
                    attn_part1(1, i)
                    attn_part2(1, i - 1)
                attn_part2(1, NP - 1)
                for g in range(NG):
                    conv3_group(1, g)

    nc.compile()
    return nc


def _prep_weights(wq, bq, wk, bk, wv, bv, wo, bo):
    import ml_dtypes

    w_all = np.concatenate([wq, wk, wv], axis=0)[:, :, 0]  # (96, 64, 3, 3)
    wpair = np.zeros((128, 3, 96), np.float32)
    wpair2 = np.zeros((128, 96), np.float32)
    wsing = np.zeros((64, 3, 96), np.float32)
    for i in range(3):  # dx = i-1 -> kx = i
        wpair[0:64, i, :] = w_all[:, :, 0, i].T  # dy=-1 -> ky=0
        wpair[64:128, i, :] = w_all[:, :, 1, i].T  # dy=0
        wsing[:, i, :] = w_all[:, :, 2, i].T  # dy=+1
    wpair2[0:64, :] = w_all[:, :, 2, 0].T   # dy=+1, dx=-1
    wpair2[64:128, :] = w_all[:, :, 2, 1].T  # dy=+1, dx=0
    wo_ = wo[:, :, 0]  # (64, 32, 3, 3)
    wo3 = np.zeros((96, 3, 64), np.float32)
    for i in range(3):
        for j in range(3):  # dy = j-1 -> ky = j
            wo3[32 * j:32 * j + 32, i, :] = wo_[:, :, j, i].T
    bqkv = np.concatenate([bq, bk, bv]).astype(np.float32)  # (96,)
    # mask/bias per chunk: phantom scan px (q%36>=32) -> 0
    maskc = np.zeros((128, NJ), np.float32)
    biasj = np.zeros((128, NJ, 96, FR), np.float32)
    for j in range(NJ):
        for p in range(128):
            m = 0.0 if ((128 * j + p) % WS) >= 32 else 1.0
            maskc[p, j] = m
            biasj[p, j, :, :] = m * bqkv[:, None]
    identb = np.eye(128).astype(ml_dtypes.bfloat16)
    return (wpair.astype(ml_dtypes.bfloat16), wpair2.astype(ml_dtypes.bfloat16),
            wsing.astype(ml_dtypes.bfloat16),
            maskc, biasj.astype(ml_dtypes.bfloat16),
            wo3.astype(ml_dtypes.bfloat16), identb)


_NC_CACHE = None


def kernel(x, wq, bq, wk, bk, wv, bv, wo, bo):
    global _NC_CACHE
    import ml_dtypes

    x = np.asarray(x, np.float32)
    # host scan buffer: xh[b,c,s,i]; xh[1 + q + 36] = x scan q (36-wide rows,
    # cols 32-35 zero); leading 1-el margin + one zero row => data rows at
    # scan rows 1..32, i.e. xh[..., 37 + 36*h + w] = x[h, w]
    xh = np.zeros((B, C, S, L1), np.float32)
    xv = xh[:, :, :, 37:37 + FSC].reshape(B, C, S, H, WS)
    xv[:, :, :, :, 0:32] = x.reshape(B, C, S, H, W)
    xh = xh.astype(ml_dtypes.bfloat16)
    wpair, wpair2, wsing, maskc, biasj, wo3, identb = _prep_weights(
        np.asarray(wq, np.float32), np.asarray(bq, np.float32),
        np.asarray(wk, np.float32), np.asarray(bk, np.float32),
        np.asarray(wv, np.float32), np.asarray(bv, np.float32),
        np.asarray(wo, np.float32), np.asarray(bo, np.float32),
    )
    bo_f = np.asarray(bo, np.float32)
    if _NC_CACHE is None:
        _NC_CACHE = build_kernel()
    nc = _NC_CACHE
    in_maps = []
    for core in range(NCORES):
        in_maps.append(
            {
                "xs": np.ascontiguousarray(xh[core * BL:(core + 1) * BL]),
                "wpair": wpair,
                "wpair2": wpair2,
                "wsing": wsing,
                "maskc": maskc,
                "biasj": biasj,
                "wo3": wo3,
                "identb": identb,
            }
        )
    res = run_bass_kernel_spmd(nc, in_maps, core_ids=list(range(NCORES)))
    outs = []
    for i in range(NCORES):
        o = np.asarray(res.results[i]["out"],
                       np.float32)  # [BL, NG, 128, NJ, FR, C]
        o = o.transpose(0, 1, 3, 2, 4, 5).reshape(BL, NG, H, WS, FR, C)
        o = o[:, :, :, 0:32]  # drop phantom cols -> [BL, g, h, w, f, C]
        o = o.transpose(0, 5, 1, 4, 2, 3)  # b, C, g, f, h, w
        outs.append(o.reshape(BL, C, S, H, W) + bo_f[None, :, None, None, None])
    return np.concatenate(outs, axis=0)


if __name__ == "__main__":
    rng = np.random.default_rng(0)
    inputs = {
        "x": rng.standard_normal((B, C, S, H, W)).astype(np.float32),
        "wq": (rng.standard_normal((Cd, C, 1, 3, 3)) * 0.04).astype(np.float32),
        "bq": (rng.standard_normal((Cd,)) * 0.04).astype(np.float32),
        "wk": (rng.standard_normal((Cd, C, 1, 3, 3)) * 0.04).astype(np.float32),
        "bk": (rng.standard_normal((Cd,)) * 0.04).astype(np.float32),
        "wv": (rng.standard_normal((Cd, C, 1, 3, 3)) * 0.04).astype(np.float32),
        "bv": (rng.standard_normal((Cd,)) * 0.04).astype(np.float32),
        "wo": (rng.standard_normal((C, Cd, 1, 3, 3)) * 0.06).astype(np.float32),
        "bo": (rng.standard_normal((C,)) * 0.06).astype(np.float32),
    }
    out = kernel(**inputs)
    print(out.shape, out.dtype)
    # quick numeric check vs numpy reference
    import numpy.lib.stride_tricks as st

    def conv3x3(xx, w, bb):
        Bn, Ci, Sn, Hn, Wn = xx.shape
        Co = w.shape[0]
        xp = np.zeros((Bn, Ci, Sn, Hn + 2, Wn + 2), np.float32)
        xp[:, :, :, 1:-1, 1:-1] = xx
        y = np.zeros((Bn, Co, Sn, Hn, Wn), np.float32)
        for ky in range(3):
            for kx in range(3):
                y += np.einsum("oi,bishw->boshw", w[:, :, 0, ky, kx],
                               xp[:, :, :, ky:ky + Hn, kx:kx + Wn])
        return y + bb[None, :, None, None, None]

    q = conv3x3(inputs["x"], inputs["wq"], inputs["bq"]).reshape(B, Cd, S, -1)
    k = conv3x3(inputs["x"], inputs["wk"], inputs["bk"]).reshape(B, Cd, S, -1)
    v = conv3x3(inputs["x"], inputs["wv"], inputs["bv"]).reshape(B, Cd, S, -1)
    att = 1.0 / (1.0 + np.exp(-np.einsum("bcsf,bctf->bcst", q, k) * SCALE))
    o = np.einsum("bcst,bctf->bcsf", att, v).reshape(B, Cd, S, H, W)
    ref = conv3x3(o, inputs["wo"], inputs["bo"])
    err = np.abs(out - ref).max() / np.abs(ref).max()
    print("rel err vs numpy ref:", err)


# revision 78
# speedup vs baseline: 1.0078x; 1.0078x over previous
"""Trainium2 Bass kernel for nn_Attention_33741263077435.

Reference computation (per batch b):
  q/k/v = conv2d_3x3(x, w{q,k,v}) + b{q,k,v}   (C=64 -> Cd=32, per frame s)
  attn  = sigmoid((q @ k^T) / 32)  per (b, channel)    (S=64, f=H*W)
  out   = attn @ v
  y     = conv2d_3x3(out, wo) + bo             (Cd=32 -> C=64)

Sharding: data-parallel over batch B=16 across 8 cores (2 batch elems/core).

Design (cost model: matmul cost = N_free rows x 0.42ns; K and M are free):
  - Frames are stored as a contiguous 36-wide scan (32 data cols + 4 zero
    pad cols shared between rows; 1152 = 9*128 scan px/frame). Both convs
    run "flipped": stationary = scan window [K=(ch,shift-copy), M=128
    contiguous scan px], moving = weights [K, N=ch_out]. A contiguous
    window is a legal 1-free-dim stationary AP, M=128 is full, and the
    shared pads make edge outputs exact.
  - Conv1 runs 5 passes per (frame, chunk): 3 row-pair passes on the
    [A;A+36] tile (dy=-1&0 x dx), 1 col-pair pass on [A+72;A+73]
    (dy=+1, dx=-1&0), 1 K=64 single (dy=+1,dx=+1): 553K PE cyc/core
    (direct normal-orientation formulation: 786K). Conv3 uses one K=96
    pass per dx over 3 row-shifted copies: 221K cyc (was 393K).
  - The 4 phantom px per row are zeroed during conv1 evacuation via the
    per-partition mask operand of scalar_tensor_tensor (out = ps*mask +
    mask*bias); zero phantoms in q/k/v are exact for attention (logits
    sum them as 0) and phantom columns of attn-out are exactly the zero
    pad cols conv3 needs.
  - Conv1 evacuations (DVE) write s-strided columns directly into an
    SBUF-resident qkv staging tile [128 f-scan, 9 j, 96 ch, 64 s]: no HBM
    qkv round trip and no q/k PE transposes (logits matmuls read staging
    views directly).
  - attn@v uses a block-diagonal att stationary [(2c,t), (2c,s)] so M=128
    covers a whole channel pair in 3 matmuls; v is pivoted to [(c,s), f]
    via 9 PE transposes/pair. Attention pairs are software-pipelined
    (attn@v of pair i emitted after logits of pair i+1) so PE never waits
    on the sigmoid.
  - Flipped conv3 writes a pixel-major bf16 HBM layout [b, g, px, j, f,
    C]; the host untransposes, casts to fp32 and adds the output bias.
  - x is loaded bf16 from a host-prepadded scan buffer (4 shifted window
    copies per group); GPSIMD cannot touch PSUM, so all PSUM evacuations
    sit on DVE/ACT while Pool handles SWDGE output/attn-staging writes
    (keeping stores off the load queues avoids head-of-line blocking).
  - Emission order: conv1(b0); attn(b0); conv1(b1) with 10 conv3(b0)
    groups hoisted into its DMA slack; remaining conv3(b0)+attn(b1)
    interleaved; conv3(b1). Per-batch-elem attn staging tensors avoid
    false whole-tensor RAW deps.
"""

import os
import sys

import numpy as np

for _p in ("/opt/trn_rl_repo", "/root/.axon_site/_ro/trn_rl_repo"):
    if os.path.isdir(_p) and _p not in sys.path:
        sys.path.append(_p)

import concourse.bass as bass  # noqa: E402
import concourse.tile as tile  # noqa: E402
from concourse import bacc, mybir  # noqa: E402
from concourse.bass_utils import run_bass_kernel_spmd  # noqa: E402

F32 = mybir.dt.float32
BF16 = mybir.dt.bfloat16

B, C, S, H, W = 16, 64, 64, 32, 32
Cd = C // 2
NCORES = 8
BL = B // NCORES
SCALE = 1.0 / np.sqrt(H * W)
FR = 4    # frames per group
NG = S // FR  # 16 groups
WS = 36   # scan row width (32 data + 4 shared zero pads)
FSC = WS * H        # 1152 scan px per frame
NJ = FSC // 128     # 9 chunks of 128 scan px
L1 = 1264           # conv1 source scan length per frame (with margins)
LP = 1228           # conv1 pad tile cols (window reads < 1228)
L3 = 1226           # attn scan staging length (37 + 1152 + 37)
LP3 = 1154          # conv3 pad tile cols
SIG = mybir.ActivationFunctionType.Sigmoid
ADD = mybir.AluOpType.add
MULT = mybir.AluOpType.mult


def build_kernel():
    nc = bacc.Bacc("TRN2", target_bir_lowering=False, debug=False)

    xs = nc.dram_tensor("xs", [BL, C, S, L1], BF16, kind="ExternalInput")
    wpair = nc.dram_tensor("wpair", [128, 3, 3 * Cd], BF16, kind="ExternalInput")
    wpair2 = nc.dram_tensor("wpair2", [128, 3 * Cd], BF16, kind="ExternalInput")
    wsing = nc.dram_tensor("wsing", [64, 3, 3 * Cd], BF16, kind="ExternalInput")
    maskc = nc.dram_tensor("maskc", [128, NJ], F32, kind="ExternalInput")
    biasj = nc.dram_tensor("biasj", [128, NJ, 3 * Cd, FR], BF16,
                           kind="ExternalInput")
    wo3 = nc.dram_tensor("wo3", [96, 3, C], BF16, kind="ExternalInput")
    identb = nc.dram_tensor("identb", [128, 128], BF16, kind="ExternalInput")

    # pixel-major output: [b, group, px, j, f, C]; host untransposes
    out_d = nc.dram_tensor("out", [BL, NG, 128, NJ, FR, C], BF16,
                           kind="ExternalOutput")
    # one staging tensor per batch elem: avoids false whole-tensor deps
    # between attn(b1) writes and conv3(b0) reads
    attn_ds = [nc.dram_tensor(f"attn_st{b}", [Cd, S, L3], BF16, kind="Internal")
               for b in range(BL)]

    with tile.TileContext(nc) as tc:
        from contextlib import ExitStack

        with ExitStack() as ctx:
            consts = ctx.enter_context(tc.tile_pool(name="consts", bufs=1))
            wp_sb = consts.tile([128, 3, 3 * Cd], BF16)
            nc.gpsimd.dma_start(wp_sb[:], wpair[:, :, :])
            wp2_sb = consts.tile([128, 3 * Cd], BF16)
            nc.gpsimd.dma_start(wp2_sb[:], wpair2[:, :])
            ws_sb = consts.tile([64, 3, 3 * Cd], BF16)
            nc.gpsimd.dma_start(ws_sb[:], wsing[:, :, :])
            mask_sb = consts.tile([128, NJ], F32)
            nc.gpsimd.dma_start(mask_sb[:], maskc[:, :])
            bj_sb = consts.tile([128, NJ, 3 * Cd, FR], BF16)
            nc.gpsimd.dma_start(bj_sb[:], biasj[:, :, :, :])
            wo_sb = consts.tile([96, 3, C], BF16)
            nc.gpsimd.dma_start(wo_sb[:], wo3[:, :, :])
            idb_sb = consts.tile([128, 128], BF16)
            nc.gpsimd.dma_start(idb_sb[:], identb[:, :])

            # qkv staging: [128 scan-part, j, ch(q|k|v), s] bf16, SBUF-resident
            stage = ctx.enter_context(tc.tile_pool(name="stage", bufs=1))
            QKV = stage.tile([128, NJ, 3 * Cd, S], BF16)
            # block-diag att tiles: off-diag blocks zeroed once, reused
            att_bufs = [stage.tile([128, 128], BF16, name=f"attbd{i}")
                        for i in range(2)]
            for a in att_bufs:
                nc.gpsimd.memset(a[0:64, 64:128], 0.0)
                nc.gpsimd.memset(a[64:128, 0:64], 0.0)
            # attn evac tiles with persistent zero margins
            evO_bufs = [stage.tile([128, L3], BF16, name=f"evo{i}")
                        for i in range(2)]
            for a in evO_bufs:
                nc.gpsimd.memset(a[:, 0:37], 0.0)
                nc.gpsimd.memset(a[:, 37 + FSC:L3], 0.0)

            with (
                tc.tile_pool(name="pad1", bufs=2) as pad_pool,
                tc.tile_pool(name="pad2", bufs=2) as pad2_pool,
                tc.tile_pool(name="psc", bufs=3, space="PSUM") as ps_pool,
                tc.tile_pool(name="v2", bufs=2) as v2_pool,
                tc.tile_pool(name="psT", bufs=2, space="PSUM") as psT_pool,
                tc.tile_pool(name="psA", bufs=1, space="PSUM") as psA_pool,
                tc.tile_pool(name="psO", bufs=2, space="PSUM") as psO_pool,
                tc.tile_pool(name="pad3", bufs=3) as pad3_pool,
                tc.tile_pool(name="oev", bufs=3) as oev_pool,
            ):

                def conv1_group(b, g, split=False):
                    s0 = FR * g
                    pad = pad_pool.tile([128, FR, LP3], BF16)
                    pad2 = pad2_pool.tile([128, FR, LP3], BF16)
                    # frame ranges: first group loads frame 0 separately so
                    # its matmuls start before the full group lands
                    franges = [(0, 1), (1, FR)] if split else [(0, FR)]
                    for (f0, f1) in franges:
                        # A half (parts 0-63): pad[i] = xscan[i-37] (dy=-1)
                        nc.sync.dma_start(pad[0:64, f0:f1, :],
                                          xs[b, :, s0 + f0:s0 + f1, 0:LP3])
                        # B half: pad[i] = xscan[i-1]  (dy=0 role)
                        nc.sync.dma_start(pad[64:128, f0:f1, :],
                                          xs[b, :, s0 + f0:s0 + f1,
                                             36:36 + LP3])
                        # pad2: [A+72 ; A+73] -> (dy=+1,dx=-1)&(dy=+1,dx=0)
                        nc.sync.dma_start(pad2[0:64, f0:f1, :],
                                          xs[b, :, s0 + f0:s0 + f1,
                                             72:72 + LP3])
                        nc.sync.dma_start(pad2[64:128, f0:f1, :],
                                          xs[b, :, s0 + f0:s0 + f1,
                                             73:73 + LP3])
                    for j in range(NJ):
                        m0 = 128 * j
                        ps = ps_pool.tile([128, FR, 3 * Cd], F32, tag="conv")
                        for f in range(FR):
                            for i in range(3):
                                # pair pass: A->dy=-1, B->dy=0, dx=i-1
                                nc.tensor.matmul(
                                    ps[:, f, :],
                                    pad[0:128, f, m0 + i:m0 + i + 128],
                                    wp_sb[:, i, :],
                                    start=(i == 0), stop=False,
                                )
                            # pair pass: dy=+1, dx=-1 & dx=0
                            nc.tensor.matmul(
                                ps[:, f, :],
                                pad2[0:128, f, m0:m0 + 128],
                                wp2_sb[:, :],
                                start=False, stop=False,
                            )
                            # single: dy=+1, dx=+1 (pad2 A-half at +2)
                            nc.tensor.matmul(
                                ps[:, f, :],
                                pad2[0:64, f, m0 + 2:m0 + 130],
                                ws_sb[:, 2, :],
                                start=False, stop=True,
                            )
                        nc.vector.scalar_tensor_tensor(
                            QKV[:, j, :, s0:s0 + FR],
                            ps[:, :, :].rearrange("p f n -> p n f"),
                            mask_sb[:, j:j + 1],
                            bj_sb[:, j, :, :],
                            op0=MULT, op1=ADD,
                        )

                attn_v2 = {}

                def attn_part1(b, cp):
                    c0 = 2 * cp
                    # v pivot: [f', (c,s)] -> [(c,s), f'] via PE transposes
                    v2 = v2_pool.tile([128, FSC], BF16)
                    for hf in range(3):
                        nch = 4 if hf < 2 else 1
                        psT = psT_pool.tile([128, 512], BF16, tag="psT")
                        for i in range(nch):
                            j = 4 * hf + i
                            nc.tensor.transpose(
                                psT[:, 128 * i:128 * i + 128],
                                QKV[:, j, 2 * Cd + c0:2 * Cd + c0 + 2, :],
                                idb_sb[:],
                            )
                        nc.vector.tensor_copy(
                            v2[:, 512 * hf:512 * hf + 128 * nch],
                            psT[:, 0:128 * nch])
                    # logits^T accumulate over f' chunks
                    psA = psA_pool.tile([128, 128], F32)
                    for j in range(NJ):
                        nc.tensor.matmul(
                            psA[:],
                            QKV[:, j, Cd + c0:Cd + c0 + 2, :],
                            QKV[:, j, c0:c0 + 2, :],
                            start=(j == 0), stop=(j == NJ - 1),
                        )
                    att = att_bufs[cp % 2]
                    nc.scalar.activation(att[0:64, 0:64], psA[0:64, 0:64], SIG,
                                         scale=float(SCALE))
                    nc.scalar.activation(att[64:128, 64:128], psA[64:128, 64:128],
                                         SIG, scale=float(SCALE))
                    attn_v2[(b, cp)] = v2

                def attn_part2(b, cp):
                    c0 = 2 * cp
                    v2 = attn_v2.pop((b, cp))
                    att = att_bufs[cp % 2]
                    evO = evO_bufs[cp % 2]
                    for t3 in range(3):
                        psO = psO_pool.tile([128, 384], F32)
                        nc.tensor.matmul(
                            psO[:], att[:, :], v2[:, 384 * t3:384 * t3 + 384],
                            start=True, stop=True,
                        )
                        nc.scalar.add(
                            evO[:, 37 + 384 * t3:37 + 384 * t3 + 384],
                            psO[:], 0.0)
                    nc.gpsimd.dma_start(
                        attn_ds[b][c0:c0 + 2, :, :].rearrange("c s l -> (c s) l"),
                        evO[:],
                    )

                def conv3_group(b, g):
                    s0 = FR * g
                    pad3 = pad3_pool.tile([96, FR, LP3], BF16)
                    src = attn_ds[b][:, s0:s0 + FR, :]
                    nc.sync.dma_start(pad3[0:32, :, :], src[:, :, 0:LP3])
                    nc.sync.dma_start(pad3[32:64, :, :], src[:, :, 36:36 + LP3])
                    nc.sync.dma_start(pad3[64:96, :, :], src[:, :, 72:72 + LP3])
                    oev = oev_pool.tile([128, NJ, FR, C], BF16)
                    for j in range(NJ):
                        m0 = 128 * j
                        ps = ps_pool.tile([128, FR, C], F32, tag="conv")
                        for f in range(FR):
                            for i in range(3):
                                nc.tensor.matmul(
                                    ps[:, f, :],
                                    pad3[0:96, f, m0 + i:m0 + i + 128],
                                    wo_sb[:, i, :],
                                    start=(i == 0), stop=(i == 2),
                                )
                        # bias bo is added on the host during un-transpose
                        if j % 2 == 0:
                            nc.scalar.add(oev[:, j, :, :], ps[:, :, :], 0.0)
                        else:
                            nc.vector.tensor_copy(oev[:, j, :, :], ps[:, :, :])
                    # Pool/SWDGE queue: empty, so waiting on oev evacs here
                    # does not head-block any loads
                    nc.gpsimd.dma_start(out_d[b, g], oev[:, :, :, :])

                NP = Cd // 2  # 16 channel pairs
                conv1_group(0, 0, split=True)
                for g in range(1, NG):
                    conv1_group(0, g)
                attn_part1(0, 0)
                for cp in range(1, NP):
                    attn_part1(0, cp)
                    attn_part2(0, cp - 1)
                attn_part2(0, NP - 1)
                # hoist 4 conv3(b0) groups into the PE-bound conv1(b1)
                # phase (its DMA has slack); rest interleave with attn(b1)
                HOIST = 10
                hoist_at = {1: 0, 3: 1, 4: 2, 6: 3, 8: 4, 9: 5, 11: 6, 12: 7,
                            14: 8, 15: 9}
                for g in range(NG):
                    conv1_group(1, g)
                    if g in hoist_at:
                        conv3_group(0, hoist_at[g])
                # spread the remaining conv3(b0) groups across the 16
                # attn(b1) pair slots
                d_at = {0: 10, 3: 11, 6: 12, 9: 13, 12: 14, 15: 15}
                if 0 in d_at:
                    conv3_group(0, d_at[0])
                attn_part1(1, 0)
                for i in range(1, NG):
                    if i in d_at:
                        conv3_group(0, d_at[i])
                    attn_part1(1, i)
                    attn_part2(1, i - 1)
                attn_part2(1, NP - 1)
                for g in range(NG):
                    conv3_group(1, g)

    nc.compile()
    return nc


def _prep_weights(wq, bq, wk, bk, wv, bv, wo, bo):
    import ml_dtypes

    w_all = np.concatenate([wq, wk, wv], axis=0)[:, :, 0]  # (96, 64, 3, 3)
    wpair = np.zeros((128, 3, 96), np.float32)
    wpair2 = np.zeros((128, 96), np.float32)
    wsing = np.zeros((64, 3, 96), np.float32)
    for i in range(3):  # dx = i-1 -> kx = i
        wpair[0:64, i, :] = w_all[:, :, 0, i].T  # dy=-1 -> ky=0
        wpair[64:128, i, :] = w_all[:, :, 1, i].T  # dy=0
        wsing[:, i, :] = w_all[:, :, 2, i].T  # dy=+1
    wpair2[0:64, :] = w_all[:, :, 2, 0].T   # dy=+1, dx=-1
    wpair2[64:128, :] = w_all[:, :, 2, 1].T  # dy=+1, dx=0
    wo_ = wo[:, :, 0]  # (64, 32, 3, 3)
    wo3 = np.zeros((96, 3, 64), np.float32)
    for i in range(3):
        for j in range(3):  # dy = j-1 -> ky = j
            wo3[32 * j:32 * j + 32, i, :] = wo_[:, :, j, i].T
    bqkv = np.concatenate([bq, bk, bv]).astype(np.float32)  # (96,)
    # mask/bias per chunk: phantom scan px (q%36>=32) -> 0
    maskc = np.zeros((128, NJ), np.float32)
    biasj = np.zeros((128, NJ, 96, FR), np.float32)
    for j in range(NJ):
        for p in range(128):
            m = 0.0 if ((128 * j + p) % WS) >= 32 else 1.0
            maskc[p, j] = m
            biasj[p, j, :, :] = m * bqkv[:, None]
    identb = np.eye(128).astype(ml_dtypes.bfloat16)
    return (wpair.astype(ml_dtypes.bfloat16), wpair2.astype(ml_dtypes.bfloat16),
            wsing.astype(ml_dtypes.bfloat16),
            maskc, biasj.astype(ml_dtypes.bfloat16),
            wo3.astype(ml_dtypes.bfloat16), identb)


_NC_CACHE = None


def kernel(x, wq, bq, wk, bk, wv, bv, wo, bo):
    global _NC_CACHE
    import ml_dtypes

    x = np.asarray(x, np.float32)
    # host scan buffer: xh[b,c,s,i]; xh[1 + q + 36] = x scan q (36-wide rows,
    # cols 32-35 zero); leading 1-el margin + one zero row => data rows at
    # scan rows 1..32, i.e. xh[..., 37 + 36*h + w] = x[h, w]
    xh = np.zeros((B, C, S, L1), np.float32)
    xv = xh[:, :, :, 37:37 + FSC].reshape(B, C, S, H, WS)
    xv[:, :, :, :, 0:32] = x.reshape(B, C, S, H, W)
    xh = xh.astype(ml_dtypes.bfloat16)
    wpair, wpair2, wsing, maskc, biasj, wo3, identb = _prep_weights(
        np.asarray(wq, np.float32), np.asarray(bq, np.float32),
        np.asarray(wk, np.float32), np.asarray(bk, np.float32),
        np.asarray(wv, np.float32), np.asarray(bv, np.float32),
        np.asarray(wo, np.float32), np.asarray(bo, np.float32),
    )
    bo_f = np.asarray(bo, np.float32)
    if _NC_CACHE is None:
        _NC_CACHE = build_kernel()
    nc = _NC_CACHE
    in_maps = []
    for core in range(NCORES):
        in_maps.append(
            {
                "xs": np.ascontiguousarray(xh[core * BL:(core + 1) * BL]),
                "wpair": wpair,
                "wpair2": wpair2,
                "wsing": wsing,
                "maskc": maskc,
                "biasj": biasj,
                "wo3": wo3,
                "identb": identb,
            }
        )
    res = run_bass_kernel_spmd(nc, in_maps, core_ids=list(range(NCORES)))
    outs = []
    for i in range(NCORES):
        o = np.asarray(res.results[i]["out"],
                       np.float32)  # [BL, NG, 128, NJ, FR, C]
        o = o.transpose(0, 1, 3, 2, 4, 5).reshape(BL, NG, H, WS, FR, C)
        o = o[:, :, :, 0:32]  # drop phantom cols -> [BL, g, h, w, f, C]
        o = o.transpose(0, 5, 1, 4, 2, 3)  # b, C, g, f, h, w
        outs.append(o.reshape(BL, C, S, H, W) + bo_f[None, :, None, None, None])
    return np.concatenate(outs, axis=0)


if __name__ == "__main__":
    rng = np.random.default_rng(0)
    inputs = {
        "x": rng.standard_normal((B, C, S, H, W)).astype(np.float32),
        "wq": (rng.standard_normal((Cd, C, 1, 3, 3)) * 0.04).astype(np.float32),
        "bq": (rng.standard_normal((Cd,)) * 0.04).astype(np.float32),
        "wk": (rng.standard_normal((Cd, C, 1, 3, 3)) * 0.04).astype(np.float32),
        "bk": (rng.standard_normal((Cd,)) * 0.04).astype(np.float32),
        "wv": (rng.standard_normal((Cd, C, 1, 3, 3)) * 0.04).astype(np.float32),
        "bv": (rng.standard_normal((Cd,)) * 0.04).astype(np.float32),
        "wo": (rng.standard_normal((C, Cd, 1, 3, 3)) * 0.06).astype(np.float32),
        "bo": (rng.standard_normal((C,)) * 0.06).astype(np.float32),
    }
    out = kernel(**inputs)
    print(out.shape, out.dtype)
    # quick numeric check vs numpy reference
    import numpy.lib.stride_tricks as st

    def conv3x3(xx, w, bb):
        Bn, Ci, Sn, Hn, Wn = xx.shape
        Co = w.shape[0]
        xp = np.zeros((Bn, Ci, Sn, Hn + 2, Wn + 2), np.float32)
        xp[:, :, :, 1:-1, 1:-1] = xx
        y = np.zeros((Bn, Co, Sn, Hn, Wn), np.float32)
        for ky in range(3):
            for kx in range(3):
                y += np.einsum("oi,bishw->boshw", w[:, :, 0, ky, kx],
                               xp[:, :, :, ky:ky + Hn, kx:kx + Wn])
        return y + bb[None, :, None, None, None]

    q = conv3x3(inputs["x"], inputs["wq"], inputs["bq"]).reshape(B, Cd, S, -1)
    k = conv3x3(inputs["x"], inputs["wk"], inputs["bk"]).reshape(B, Cd, S, -1)
    v = conv3x3(inputs["x"], inputs["wv"], inputs["bv"]).reshape(B, Cd, S, -1)
    att = 1.0 / (1.0 + np.exp(-np.einsum("bcsf,bctf->bcst", q, k) * SCALE))
    o = np.einsum("bcst,bctf->bcsf", att, v).reshape(B, Cd, S, H, W)
    ref = conv3x3(o, inputs["wo"], inputs["bo"])
    err = np.abs(out - ref).max() / np.abs(ref).max()
    print("rel err vs numpy ref:", err)


# revision 87
# speedup vs baseline: 1.0146x; 1.0068x over previous
"""Trainium2 Bass kernel for nn_Attention_33741263077435.

Reference computation (per batch b):
  q/k/v = conv2d_3x3(x, w{q,k,v}) + b{q,k,v}   (C=64 -> Cd=32, per frame s)
  attn  = sigmoid((q @ k^T) / 32)  per (b, channel)    (S=64, f=H*W)
  out   = attn @ v
  y     = conv2d_3x3(out, wo) + bo             (Cd=32 -> C=64)

Sharding: data-parallel over batch B=16 across 8 cores (2 batch elems/core).

Design (cost model: matmul cost = N_free rows x 0.42ns; K and M are free):
  - Frames are stored as a contiguous 36-wide scan (32 data cols + 4 zero
    pad cols shared between rows; 1152 = 9*128 scan px/frame). Both convs
    run "flipped": stationary = scan window [K=(ch,shift-copy), M=128
    contiguous scan px], moving = weights [K, N=ch_out]. A contiguous
    window is a legal 1-free-dim stationary AP, M=128 is full, and the
    shared pads make edge outputs exact.
  - Conv1 runs 5 passes per (frame, chunk): 3 row-pair passes on the
    [A;A+36] tile (dy=-1&0 x dx), 1 col-pair pass on [A+72;A+73]
    (dy=+1, dx=-1&0), 1 K=64 single (dy=+1,dx=+1): 553K PE cyc/core
    (direct normal-orientation formulation: 786K). Conv3 uses one K=96
    pass per dx over 3 row-shifted copies: 221K cyc (was 393K).
  - The 4 phantom px per row are zeroed during conv1 evacuation via the
    per-partition mask operand of scalar_tensor_tensor (out = ps*mask +
    mask*bias); zero phantoms in q/k/v are exact for attention (logits
    sum them as 0) and phantom columns of attn-out are exactly the zero
    pad cols conv3 needs.
  - Conv1 evacuations (DVE) write s-strided columns directly into an
    SBUF-resident qkv staging tile [128 f-scan, 9 j, 96 ch, 64 s]: no HBM
    qkv round trip and no q/k PE transposes (logits matmuls read staging
    views directly).
  - attn@v uses a block-diagonal att stationary [(2c,t), (2c,s)] so M=128
    covers a whole channel pair in 3 matmuls; v is pivoted to [(c,s), f]
    via 9 PE transposes/pair. Attention pairs are software-pipelined
    (attn@v of pair i emitted after logits of pair i+1) so PE never waits
    on the sigmoid.
  - Flipped conv3 writes a pixel-major bf16 HBM layout [b, g, px, j, f,
    C]; the host untransposes, casts to fp32 and adds the output bias.
  - x is loaded bf16 from a host-prepadded scan buffer (4 shifted window
    copies per group); GPSIMD cannot touch PSUM, so all PSUM evacuations
    sit on DVE/ACT while Pool handles SWDGE output/attn-staging writes
    (keeping stores off the load queues avoids head-of-line blocking).
  - Emission order: conv1(b0); attn(b0); conv1(b1) with 10 conv3(b0)
    groups hoisted into its DMA slack; remaining conv3(b0)+attn(b1)
    interleaved; conv3(b1). Per-batch-elem attn staging tensors avoid
    false whole-tensor RAW deps.
"""

import os
import sys

import numpy as np

for _p in ("/opt/trn_rl_repo", "/root/.axon_site/_ro/trn_rl_repo"):
    if os.path.isdir(_p) and _p not in sys.path:
        sys.path.append(_p)

import concourse.bass as bass  # noqa: E402
import concourse.tile as tile  # noqa: E402
from concourse import bacc, mybir  # noqa: E402
from concourse.bass_utils import run_bass_kernel_spmd  # noqa: E402

F32 = mybir.dt.float32
BF16 = mybir.dt.bfloat16

B, C, S, H, W = 16, 64, 64, 32, 32
Cd = C // 2
NCORES = 8
BL = B // NCORES
SCALE = 1.0 / np.sqrt(H * W)
FR = 4    # frames per group
NG = S // FR  # 16 groups
WS = 36   # scan row width (32 data + 4 shared zero pads)
FSC = WS * H        # 1152 scan px per frame
NJ = FSC // 128     # 9 chunks of 128 scan px
L1 = 1264           # conv1 source scan length per frame (with margins)
LP = 1228           # conv1 pad tile cols (window reads < 1228)
L3 = 1226           # attn scan staging length (37 + 1152 + 37)
LP3 = 1154          # conv3 pad tile cols
SIG = mybir.ActivationFunctionType.Sigmoid
ADD = mybir.AluOpType.add
MULT = mybir.AluOpType.mult


def build_kernel():
    nc = bacc.Bacc("TRN2", target_bir_lowering=False, debug=False)

    xs = nc.dram_tensor("xs", [BL, C, S, L1], BF16, kind="ExternalInput")
    wpair = nc.dram_tensor("wpair", [128, 3, 3 * Cd], BF16, kind="ExternalInput")
    wpair2 = nc.dram_tensor("wpair2", [128, 3 * Cd], BF16, kind="ExternalInput")
    wsing = nc.dram_tensor("wsing", [64, 3, 3 * Cd], BF16, kind="ExternalInput")
    maskc = nc.dram_tensor("maskc", [128, NJ], F32, kind="ExternalInput")
    biasj = nc.dram_tensor("biasj", [128, NJ, 3 * Cd, FR], BF16,
                           kind="ExternalInput")
    wo3 = nc.dram_tensor("wo3", [96, 3, C], BF16, kind="ExternalInput")
    identb = nc.dram_tensor("identb", [128, 128], BF16, kind="ExternalInput")

    # pixel-major output: [b, group, px, j, f, C]; host untransposes
    out_d = nc.dram_tensor("out", [BL, NG, 128, NJ, FR, C], BF16,
                           kind="ExternalOutput")
    # one staging tensor per batch elem: avoids false whole-tensor deps
    # between attn(b1) writes and conv3(b0) reads
    attn_ds = [nc.dram_tensor(f"attn_st{b}", [Cd, S, L3], BF16, kind="Internal")
               for b in range(BL)]

    with tile.TileContext(nc) as tc:
        from contextlib import ExitStack

        with ExitStack() as ctx:
            consts = ctx.enter_context(tc.tile_pool(name="consts", bufs=1))
            wp_sb = consts.tile([128, 3, 3 * Cd], BF16)
            nc.sync.dma_start(wp_sb[:], wpair[:, :, :])
            wp2_sb = consts.tile([128, 3 * Cd], BF16)
            nc.sync.dma_start(wp2_sb[:], wpair2[:, :])
            ws_sb = consts.tile([64, 3, 3 * Cd], BF16)
            nc.sync.dma_start(ws_sb[:], wsing[:, :, :])
            mask_sb = consts.tile([128, NJ], F32)
            nc.gpsimd.dma_start(mask_sb[:], maskc[:, :])
            bj_sb = consts.tile([128, NJ, 3 * Cd, FR], BF16)
            nc.gpsimd.dma_start(bj_sb[:], biasj[:, :, :, :])
            wo_sb = consts.tile([96, 3, C], BF16)
            nc.gpsimd.dma_start(wo_sb[:], wo3[:, :, :])
            idb_sb = consts.tile([128, 128], BF16)
            nc.gpsimd.dma_start(idb_sb[:], identb[:, :])

            # qkv staging: [128 scan-part, j, ch(q|k|v), s] bf16, SBUF-resident
            stage = ctx.enter_context(tc.tile_pool(name="stage", bufs=1))
            QKV = stage.tile([128, NJ, 3 * Cd, S], BF16)
            # block-diag att tiles: off-diag blocks zeroed once, reused
            att_bufs = [stage.tile([128, 128], BF16, name=f"attbd{i}")
                        for i in range(2)]
            for a in att_bufs:
                nc.gpsimd.memset(a[0:64, 64:128], 0.0)
                nc.gpsimd.memset(a[64:128, 0:64], 0.0)
            # attn evac tiles with persistent zero margins
            evO_bufs = [stage.tile([128, L3], BF16, name=f"evo{i}")
                        for i in range(2)]
            for a in evO_bufs:
                nc.gpsimd.memset(a[:, 0:37], 0.0)
                nc.gpsimd.memset(a[:, 37 + FSC:L3], 0.0)

            with (
                tc.tile_pool(name="pad1", bufs=2) as pad_pool,
                tc.tile_pool(name="pad2", bufs=2) as pad2_pool,
                tc.tile_pool(name="psc", bufs=3, space="PSUM") as ps_pool,
                tc.tile_pool(name="v2", bufs=2) as v2_pool,
                tc.tile_pool(name="psT", bufs=2, space="PSUM") as psT_pool,
                tc.tile_pool(name="psA", bufs=1, space="PSUM") as psA_pool,
                tc.tile_pool(name="psO", bufs=2, space="PSUM") as psO_pool,
                tc.tile_pool(name="pad3", bufs=3) as pad3_pool,
                tc.tile_pool(name="oev", bufs=3) as oev_pool,
            ):

                def conv1_group(b, g, split=False):
                    s0 = FR * g
                    pad = pad_pool.tile([128, FR, LP3], BF16)
                    pad2 = pad2_pool.tile([128, FR, LP3], BF16)
                    # frame ranges: first group loads frame 0 separately so
                    # its matmuls start before the full group lands
                    franges = [(0, 1), (1, FR)] if split else [(0, FR)]
                    for (f0, f1) in franges:
                        # A half (parts 0-63): pad[i] = xscan[i-37] (dy=-1)
                        nc.sync.dma_start(pad[0:64, f0:f1, :],
                                          xs[b, :, s0 + f0:s0 + f1, 0:LP3])
                        # B half: pad[i] = xscan[i-1]  (dy=0 role)
                        nc.sync.dma_start(pad[64:128, f0:f1, :],
                                          xs[b, :, s0 + f0:s0 + f1,
                                             36:36 + LP3])
                        # pad2: [A+72 ; A+73] -> (dy=+1,dx=-1)&(dy=+1,dx=0)
                        nc.sync.dma_start(pad2[0:64, f0:f1, :],
                                          xs[b, :, s0 + f0:s0 + f1,
                                             72:72 + LP3])
                        nc.sync.dma_start(pad2[64:128, f0:f1, :],
                                          xs[b, :, s0 + f0:s0 + f1,
                                             73:73 + LP3])
                    for j in range(NJ):
                        m0 = 128 * j
                        ps = ps_pool.tile([128, FR, 3 * Cd], F32, tag="conv")
                        for f in range(FR):
                            for i in range(3):
                                # pair pass: A->dy=-1, B->dy=0, dx=i-1
                                nc.tensor.matmul(
                                    ps[:, f, :],
                                    pad[0:128, f, m0 + i:m0 + i + 128],
                                    wp_sb[:, i, :],
                                    start=(i == 0), stop=False,
                                )
                            # pair pass: dy=+1, dx=-1 & dx=0
                            nc.tensor.matmul(
                                ps[:, f, :],
                                pad2[0:128, f, m0:m0 + 128],
                                wp2_sb[:, :],
                                start=False, stop=False,
                            )
                            # single: dy=+1, dx=+1 (pad2 A-half at +2)
                            nc.tensor.matmul(
                                ps[:, f, :],
                                pad2[0:64, f, m0 + 2:m0 + 130],
                                ws_sb[:, 2, :],
                                start=False, stop=True,
                            )
                        nc.vector.scalar_tensor_tensor(
                            QKV[:, j, :, s0:s0 + FR],
                            ps[:, :, :].rearrange("p f n -> p n f"),
                            mask_sb[:, j:j + 1],
                            bj_sb[:, j, :, :],
                            op0=MULT, op1=ADD,
                        )

                attn_v2 = {}

                def attn_part1(b, cp):
                    c0 = 2 * cp
                    # v pivot: [f', (c,s)] -> [(c,s), f'] via PE transposes
                    v2 = v2_pool.tile([128, FSC], BF16)
                    for hf in range(3):
                        nch = 4 if hf < 2 else 1
                        psT = psT_pool.tile([128, 512], BF16, tag="psT")
                        for i in range(nch):
                            j = 4 * hf + i
                            nc.tensor.transpose(
                                psT[:, 128 * i:128 * i + 128],
                                QKV[:, j, 2 * Cd + c0:2 * Cd + c0 + 2, :],
                                idb_sb[:],
                            )
                        nc.vector.tensor_copy(
                            v2[:, 512 * hf:512 * hf + 128 * nch],
                            psT[:, 0:128 * nch])
                    # logits^T accumulate over f' chunks
                    psA = psA_pool.tile([128, 128], F32)
                    for j in range(NJ):
                        nc.tensor.matmul(
                            psA[:],
                            QKV[:, j, Cd + c0:Cd + c0 + 2, :],
                            QKV[:, j, c0:c0 + 2, :],
                            start=(j == 0), stop=(j == NJ - 1),
                        )
                    att = att_bufs[cp % 2]
                    nc.scalar.activation(att[0:64, 0:64], psA[0:64, 0:64], SIG,
                                         scale=float(SCALE))
                    nc.scalar.activation(att[64:128, 64:128], psA[64:128, 64:128],
                                         SIG, scale=float(SCALE))
                    attn_v2[(b, cp)] = v2

                def attn_part2(b, cp):
                    c0 = 2 * cp
                    v2 = attn_v2.pop((b, cp))
                    att = att_bufs[cp % 2]
                    evO = evO_bufs[cp % 2]
                    for t3 in range(3):
                        psO = psO_pool.tile([128, 384], F32)
                        nc.tensor.matmul(
                            psO[:], att[:, :], v2[:, 384 * t3:384 * t3 + 384],
                            start=True, stop=True,
                        )
                        nc.scalar.add(
                            evO[:, 37 + 384 * t3:37 + 384 * t3 + 384],
                            psO[:], 0.0)
                    nc.gpsimd.dma_start(
                        attn_ds[b][c0:c0 + 2, :, :].rearrange("c s l -> (c s) l"),
                        evO[:],
                    )

                def conv3_group(b, g):
                    s0 = FR * g
                    pad3 = pad3_pool.tile([96, FR, LP3], BF16)
                    src = attn_ds[b][:, s0:s0 + FR, :]
                    nc.sync.dma_start(pad3[0:32, :, :], src[:, :, 0:LP3])
                    nc.sync.dma_start(pad3[32:64, :, :], src[:, :, 36:36 + LP3])
                    nc.sync.dma_start(pad3[64:96, :, :], src[:, :, 72:72 + LP3])
                    oev = oev_pool.tile([128, NJ, FR, C], BF16)
                    for j in range(NJ):
                        m0 = 128 * j
                        ps = ps_pool.tile([128, FR, C], F32, tag="conv")
                        for f in range(FR):
                            for i in range(3):
                                nc.tensor.matmul(
                                    ps[:, f, :],
                                    pad3[0:96, f, m0 + i:m0 + i + 128],
                                    wo_sb[:, i, :],
                                    start=(i == 0), stop=(i == 2),
                                )
                        # bias bo is added on the host during un-transpose
                        if j % 2 == 0:
                            nc.scalar.add(oev[:, j, :, :], ps[:, :, :], 0.0)
                        else:
                            nc.vector.tensor_copy(oev[:, j, :, :], ps[:, :, :])
                    # Pool/SWDGE queue: empty, so waiting on oev evacs here
                    # does not head-block any loads
                    if b == 1 and g == NG - 1:
                        nc.gpsimd.dma_start(out_d[b, g, :, 0:5], oev[:, 0:5])
                        nc.gpsimd.dma_start(out_d[b, g, :, 5:NJ], oev[:, 5:NJ])
                    else:
                        nc.gpsimd.dma_start(out_d[b, g], oev[:, :, :, :])

                NP = Cd // 2  # 16 channel pairs
                conv1_group(0, 0, split=True)
                for g in range(1, NG):
                    conv1_group(0, g)
                attn_part1(0, 0)
                for cp in range(1, NP):
                    attn_part1(0, cp)
                    attn_part2(0, cp - 1)
                attn_part2(0, NP - 1)
                # hoist 4 conv3(b0) groups into the PE-bound conv1(b1)
                # phase (its DMA has slack); rest interleave with attn(b1)
                HOIST = 10
                hoist_at = {1: 0, 3: 1, 4: 2, 6: 3, 8: 4, 9: 5, 11: 6, 12: 7,
                            14: 8, 15: 9}
                for g in range(NG):
                    conv1_group(1, g)
                    if g in hoist_at:
                        conv3_group(0, hoist_at[g])
                # spread the remaining conv3(b0) groups across the 16
                # attn(b1) pair slots
                d_at = {0: 10, 3: 11, 6: 12, 9: 13, 12: 14, 15: 15}
                if 0 in d_at:
                    conv3_group(0, d_at[0])
                attn_part1(1, 0)
                for i in range(1, NG):
                    if i in d_at:
                        conv3_group(0, d_at[i])
                    attn_part1(1, i)
                    attn_part2(1, i - 1)
                attn_part2(1, NP - 1)
                for g in range(NG):
                    conv3_group(1, g)

    nc.compile()
    return nc


def _prep_weights(wq, bq, wk, bk, wv, bv, wo, bo):
    import ml_dtypes

    w_all = np.concatenate([wq, wk, wv], axis=0)[:, :, 0]  # (96, 64, 3, 3)
    wpair = np.zeros((128, 3, 96), np.float32)
    wpair2 = np.zeros((128, 96), np.float32)
    wsing = np.zeros((64, 3, 96), np.float32)
    for i in range(3):  # dx = i-1 -> kx = i
        wpair[0:64, i, :] = w_all[:, :, 0, i].T  # dy=-1 -> ky=0
        wpair[64:128, i, :] = w_all[:, :, 1, i].T  # dy=0
        wsing[:, i, :] = w_all[:, :, 2, i].T  # dy=+1
    wpair2[0:64, :] = w_all[:, :, 2, 0].T   # dy=+1, dx=-1
    wpair2[64:128, :] = w_all[:, :, 2, 1].T  # dy=+1, dx=0
    wo_ = wo[:, :, 0]  # (64, 32, 3, 3)
    wo3 = np.zeros((96, 3, 64), np.float32)
    for i in range(3):
        for j in range(3):  # dy = j-1 -> ky = j
            wo3[32 * j:32 * j + 32, i, :] = wo_[:, :, j, i].T
    bqkv = np.concatenate([bq, bk, bv]).astype(np.float32)  # (96,)
    # mask/bias per chunk: phantom scan px (q%36>=32) -> 0
    maskc = np.zeros((128, NJ), np.float32)
    biasj = np.zeros((128, NJ, 96, FR), np.float32)
    for j in range(NJ):
        for p in range(128):
            m = 0.0 if ((128 * j + p) % WS) >= 32 else 1.0
            maskc[p, j] = m
            biasj[p, j, :, :] = m * bqkv[:, None]
    identb = np.eye(128).astype(ml_dtypes.bfloat16)
    return (wpair.astype(ml_dtypes.bfloat16), wpair2.astype(ml_dtypes.bfloat16),
            wsing.astype(ml_dtypes.bfloat16),
            maskc, biasj.astype(ml_dtypes.bfloat16),
            wo3.astype(ml_dtypes.bfloat16), identb)


_NC_CACHE = None


def kernel(x, wq, bq, wk, bk, wv, bv, wo, bo):
    global _NC_CACHE
    import ml_dtypes

    x = np.asarray(x, np.float32)
    # host scan buffer: xh[b,c,s,i]; xh[1 + q + 36] = x scan q (36-wide rows,
    # cols 32-35 zero); leading 1-el margin + one zero row => data rows at
    # scan rows 1..32, i.e. xh[..., 37 + 36*h + w] = x[h, w]
    xh = np.zeros((B, C, S, L1), np.float32)
    xv = xh[:, :, :, 37:37 + FSC].reshape(B, C, S, H, WS)
    xv[:, :, :, :, 0:32] = x.reshape(B, C, S, H, W)
    xh = xh.astype(ml_dtypes.bfloat16)
    wpair, wpair2, wsing, maskc, biasj, wo3, identb = _prep_weights(
        np.asarray(wq, np.float32), np.asarray(bq, np.float32),
        np.asarray(wk, np.float32), np.asarray(bk, np.float32),
        np.asarray(wv, np.float32), np.asarray(bv, np.float32),
        np.asarray(wo, np.float32), np.asarray(bo, np.float32),
    )
    bo_f = np.asarray(bo, np.float32)
    if _NC_CACHE is None:
        _NC_CACHE = build_kernel()
    nc = _NC_CACHE
    in_maps = []
    for core in range(NCORES):
        in_maps.append(
            {
                "xs": np.ascontiguousarray(xh[core * BL:(core + 1) * BL]),
                "wpair": wpair,
                "wpair2": wpair2,
                "wsing": wsing,
                "maskc": maskc,
                "biasj": biasj,
                "wo3": wo3,
                "identb": identb,
            }
        )
    res = run_bass_kernel_spmd(nc, in_maps, core_ids=list(range(NCORES)))
    outs = []
    for i in range(NCORES):
        o = np.asarray(res.results[i]["out"],
                       np.float32)  # [BL, NG, 128, NJ, FR, C]
        o = o.transpose(0, 1, 3, 2, 4, 5).reshape(BL, NG, H, WS, FR, C)
        o = o[:, :, :, 0:32]  # drop phantom cols -> [BL, g, h, w, f, C]
        o = o.transpose(0, 5, 1, 4, 2, 3)  # b, C, g, f, h, w
        outs.append(o.reshape(BL, C, S, H, W) + bo_f[None, :, None, None, None])
    return np.concatenate(outs, axis=0)


if __name__ == "__main__":
    rng = np.random.default_rng(0)
    inputs = {
        "x": rng.standard_normal((B, C, S, H, W)).astype(np.float32),
        "wq": (rng.standard_normal((Cd, C, 1, 3, 3)) * 0.04).astype(np.float32),
        "bq": (rng.standard_normal((Cd,)) * 0.04).astype(np.float32),
        "wk": (rng.standard_normal((Cd, C, 1, 3, 3)) * 0.04).astype(np.float32),
        "bk": (rng.standard_normal((Cd,)) * 0.04).astype(np.float32),
        "wv": (rng.standard_normal((Cd, C, 1, 3, 3)) * 0.04).astype(np.float32),
        "bv": (rng.standard_normal((Cd,)) * 0.04).astype(np.float32),
        "wo": (rng.standard_normal((C, Cd, 1, 3, 3)) * 0.06).astype(np.float32),
        "bo": (rng.standard_normal((C,)) * 0.06).astype(np.float32),
    }
    out = kernel(**inputs)
    print(out.shape, out.dtype)
    # quick numeric check vs numpy reference
    import numpy.lib.stride_tricks as st

    def conv3x3(xx, w, bb):
        Bn, Ci, Sn, Hn, Wn = xx.shape
        Co = w.shape[0]
        xp = np.zeros((Bn, Ci, Sn, Hn + 2, Wn + 2), np.float32)
        xp[:, :, :, 1:-1, 1:-1] = xx
        y = np.zeros((Bn, Co, Sn, Hn, Wn), np.float32)
        for ky in range(3):
            for kx in range(3):
                y += np.einsum("oi,bishw->boshw", w[:, :, 0, ky, kx],
                               xp[:, :, :, ky:ky + Hn, kx:kx + Wn])
        return y + bb[None, :, None, None, None]

    q = conv3x3(inputs["x"], inputs["wq"], inputs["bq"]).reshape(B, Cd, S, -1)
    k = conv3x3(inputs["x"], inputs["wk"], inputs["bk"]).reshape(B, Cd, S, -1)
    v = conv3x3(inputs["x"], inputs["wv"], inputs["bv"]).reshape(B, Cd, S, -1)
    att = 1.0 / (1.0 + np.exp(-np.einsum("bcsf,bctf->bcst", q, k) * SCALE))
    o = np.einsum("bcst,bctf->bcsf", att, v).reshape(B, Cd, S, H, W)
    ref = conv3x3(o, inputs["wo"], inputs["bo"])
    err = np.abs(out - ref).max() / np.abs(ref).max()
    print("rel err vs numpy ref:", err)
